# revision 9
# baseline (speedup 1.0000x reference)
"""Trainium2 Bass kernel for nn_DecoderRNN (attention LSTM decoder + vocab projection).

Strategy (8 NeuronCores):
  - The 63-step LSTM/attention recurrence is replicated on all cores (identical
    SPMD program); the dominant output projection (T*B, H) x (H, V) is sharded
    over the vocab dimension (V/8 = 1250 logit columns per core). No collectives.
  - fp8-e4m3 + DoubleRow perf mode for the recurrence GEMMs (gates from x/
    attended/h).  Fold matrices Cx/Ca and the step-0 gates are precomputed on
    the host in fp32.
  - Gate columns are ordered [g|i|f|o]; each 512-wide gate chunk lives in its
    own single-bank PSUM tile.  The per-chunk gate bias is seeded by FOUR
    CONCURRENT K=1 matmuls (tile_position row strips 0/32/64/96, one PSUM bank
    each) instead of four serial N=512 streams.
  - The recurrent critical path is aggressively shortened: per-m-tile
    exp/sum pipelined inside the attention-finish GEMMs, fast approximate
    reciprocal for the softmax denominator, h-side gate GEMMs fill the
    softmax window, attention-side GEMMs close chunks in [f,o,i,g] order
    with 512-wide tanh ops pipelined per chunk.
  - h is stored DOUBLED (2h) so sigmoid folds into single fused
    scalar_tensor_tensor ops ((tanh+1)*y); the 0.5 is folded into the
    att_Wh / W_hh / out_W weights on the host.  c is stored doubled too
    (tanh(c) via the activation input scale).  All activations stay on the
    exp/tanh table: zero ACT table reloads in the loop.
  - The output projection runs on 128-row batches with a double-buffered PSUM
    bank; its PSUM->SBUF copies are deferred off the critical chain.
  - Logits are written bf16, valid rows only; the host zero-fills, upcasts,
    and adds the output bias.  Ragged lengths are baked into the instruction
    stream.
"""

import os
import sys

import numpy as np

for _p in ("/opt/trn_rl_repo", "/root/.axon_site/_ro/trn_rl_repo"):
    if os.path.isdir(_p) and _p not in sys.path:
        sys.path.insert(0, _p)

import ml_dtypes
import concourse.bass as bass
import concourse.tile as tile
from concourse import bacc, mybir
from concourse.bass_utils import run_bass_kernel_spmd
from concourse.masks import make_identity

F32 = mybir.dt.float32
BF16 = mybir.dt.bfloat16
F8 = mybir.dt.float8e4
I32 = mybir.dt.int32
ADD = mybir.AluOpType.add
MULT = mybir.AluOpType.mult
TANH = mybir.ActivationFunctionType.Tanh
EXP = mybir.ActivationFunctionType.Exp
COPY = mybir.ActivationFunctionType.Copy
DR = mybir.MatmulPerfMode.DoubleRow
NP_BF16 = ml_dtypes.bfloat16
NP_F8 = np.dtype(mybir.dt.np(F8))

B, T, E, H, A, V = 128, 64, 512, 512, 512, 10000
G4 = 4 * H                      # 2048
NCORES = 8
VS = V // NCORES                # 1250 vocab columns per core
P = 128

KE = E // P                     # 4 k-tiles over E
KH = H // P
KA = A // P
MA = A // P                     # A m-tiles (feature-major attention)
NCH = 4                         # four 512-wide gate chunks: [g|i|f|o]


def _flush_plan(n_t):
    """Pack per-step h rows into 128-row batches for the output projection."""
    plan = []          # per t: (col0, flush_before: segments or None)
    segs = []
    pos = 0
    for t in range(T):
        nt = int(n_t[t])
        flush = None
        if pos + nt > P:
            flush = segs
            segs = []
            pos = 0
        plan.append((pos, flush))
        segs.append((t, pos, pos + nt))
        pos += nt
    return plan, segs  # segs = final leftover batch


def _build_nc(n_t):
    nc = bacc.Bacc("TRN2", target_bir_lowering=False, debug=False,
                   num_devices=NCORES)

    # ---------------- I/O ----------------
    cnn_T = nc.declare_dram_parameter("cnn_T", [A, B], BF16, isOutput=False)
    caps = nc.declare_dram_parameter("caps", [T, B], I32, isOutput=False)
    emb_W = nc.declare_dram_parameter("emb_W", [V, E], BF16, isOutput=False)
    awh_d = nc.declare_dram_parameter("awh", [H, A], BF16, isOutput=False)
    awx_d = nc.declare_dram_parameter("awx", [E, A], BF16, isOutput=False)
    attb_row = nc.declare_dram_parameter("attb_row", [1, A], BF16, isOutput=False)
    cx8_d = nc.declare_dram_parameter("cx8", [E, G4], F8, isOutput=False)
    ca8_d = nc.declare_dram_parameter("ca8", [A, G4], F8, isOutput=False)
    whh8_d = nc.declare_dram_parameter("whh8", [H, G4], F8, isOutput=False)
    bc4_d = nc.declare_dram_parameter("bc4", [P, G4], BF16, isOutput=False)
    g0_d = nc.declare_dram_parameter("g0", [B, G4], F32, isOutput=False)
    owt_d = nc.declare_dram_parameter("owt", [H, VS], BF16, isOutput=False)
    out = nc.declare_dram_parameter("out", [T, B, VS], BF16, isOutput=True)

    plan, final_segs = _flush_plan(n_t)

    with tile.TileContext(nc) as tc:
        with (
            tc.tile_pool(name="consts", bufs=1) as consts,
            tc.tile_pool(name="state", bufs=1) as state,
            tc.tile_pool(name="work", bufs=2) as work,
            tc.tile_pool(name="xstream", bufs=3) as xstream,
            tc.tile_pool(name="ps_g", bufs=1, space="PSUM") as ps_g,    # 4 banks
            tc.tile_pool(name="ps_s", bufs=1, space="PSUM") as ps_s,    # 1 bank
            tc.tile_pool(name="ps_tr", bufs=1, space="PSUM") as ps_tr,  # 1 bank
            tc.tile_pool(name="ps_o", bufs=2, space="PSUM") as ps_o,    # 2 banks
        ):
            # ------------- weight / const loads (three HWDGE queues) -------------
            ident16 = consts.tile([P, P], BF16)
            make_identity(nc, ident16)
            ones_bf = consts.tile([P, 1], BF16)
            nc.vector.memset(ones_bf, 1.0)

            def load3(dst, dram_ap):
                nc.sync.dma_start(dst, dram_ap.rearrange("(k p) n -> p k n", p=P))

            def load3b(dst, dram_ap):
                nc.scalar.dma_start(dst, dram_ap.rearrange("(k p) n -> p k n", p=P))

            def load3c(dst, dram_ap):
                nc.gpsimd.dma_start(dst, dram_ap.rearrange("(k p) n -> p k n", p=P))

            cnn_sb = consts.tile([P, KA, B], BF16)
            load3(cnn_sb, cnn_T[:, :])
            attb_sb = consts.tile([1, A], BF16)
            nc.sync.dma_start(attb_sb, attb_row[:, :])
            bc4_sb = consts.tile([P, G4], BF16)
            nc.gpsimd.dma_start(bc4_sb, bc4_d[:, :])
            g0_sb = consts.tile([P, G4], F32)
            nc.sync.dma_start(g0_sb, g0_d[:, :])

            awh_sb = state.tile([P, KH, A], BF16)
            load3(awh_sb, awh_d[:, :])
            awx_sb = state.tile([P, KE, A], BF16)
            load3(awx_sb, awx_d[:, :])
            cx8_sb = state.tile([P, KE, G4], F8)
            load3b(cx8_sb, cx8_d[:, :])
            ca8_sb = state.tile([P, KA, G4], F8)
            load3b(ca8_sb, ca8_d[:, :])
            whh8_sb = state.tile([P, KH, G4], F8)
            load3b(whh8_sb, whh8_d[:, :])
            owt_sb = state.tile([P, KH, VS], BF16)
            load3c(owt_sb, owt_d[:, :])
            toks = state.tile([B, T], I32)
            nc.gpsimd.dma_start(toks, caps[:, :].rearrange("t b -> b t"))

            # recurrent state (c stored doubled; h stored doubled in stages)
            c2_sb = state.tile([P, H], BF16)
            stages = [state.tile([P, KH, P], BF16, name=f"stage{i}")
                      for i in range(2)]
            stages8 = [state.tile([P, KH, P], F8, name=f"stage8_{i}")
                       for i in range(2)]

            ones_row = ones_bf[0:1, 0:1]

            # gate chunk -> (tile index, column range); chunks are [g|i|f|o]
            CH_G, CH_I, CH_F, CH_O = 0, 1, 2, 3

            def chunk_ref(Gs, ci):
                return Gs[ci // 2], slice((ci % 2) * 512, (ci % 2 + 1) * 512)

            # ---------------- helpers ----------------
            def fetch_x(t):
                """Gather x_t embeddings; bf16 [E(part), KE, B] + fp8 cast."""
                xg = xstream.tile([P, E], BF16, tag="xg")
                nc.gpsimd.indirect_dma_start(
                    out=xg, out_offset=None, in_=emb_W[:, :],
                    in_offset=bass.IndirectOffsetOnAxis(ap=toks[:, t - 1:t], axis=0))
                xT = xstream.tile([P, KE, B], BF16, tag="xT")
                nc.sync.dma_start_transpose(xT, xg)
                x8 = xstream.tile([P, KE, B], F8, tag="x8")
                nc.vector.tensor_copy(x8, xT)
                return xT, x8

            def start_scores(t, xT):
                """New PSUM score tile for step t: att_b + PA."""
                nt = int(n_t[t])
                S = ps_s.tile([P, MA, B], F32, tag="att")
                for m in range(MA):
                    nc.tensor.matmul(S[:, m, 0:nt],
                                     attb_sb[0:1, m * P:(m + 1) * P],
                                     ones_row.to_broadcast([1, nt]),
                                     start=True, stop=False)
                    for k in range(KE):
                        nc.tensor.matmul(S[:, m, 0:nt],
                                         awx_sb[:, k, m * P:(m + 1) * P],
                                         xT[:, k, 0:nt], start=False, stop=False)
                return S

            def seed_gates(t, x8):
                """Bias via 4 concurrent row-tiled K=1 matmuls + PX (fp8 DR)."""
                nt = int(n_t[t])
                Gs = [ps_g.tile([P, 1024], F32, tag=f"g{h}", name=f"g{h}")
                      for h in range(2)]
                for ci in range(NCH):
                    Gh, rg = chunk_ref(Gs, ci)
                    rp = 32 * ci
                    nc.tensor.matmul(
                        Gh[0:nt, rg],
                        ones_bf[rp:rp + 1, 0:1].to_broadcast([1, nt]),
                        bc4_sb[rp:rp + 1, ci * 512:(ci + 1) * 512],
                        start=True, stop=False, tile_position=(rp, 0))
                for ci in range(NCH):
                    Gh, rg = chunk_ref(Gs, ci)
                    ns = slice(ci * 512, (ci + 1) * 512)
                    for j in range(KE // 2):
                        nc.tensor.matmul(Gh[0:nt, rg], x8[:, 2 * j:2 * j + 2, 0:nt],
                                         cx8_sb[:, 2 * j:2 * j + 2, ns],
                                         start=False, stop=False, perf_mode=DR)
                return Gs

            def h_side_half(t, Gs, hstage8, hcol, chunks):
                """+= h_{t-1} @ W_hh.T for the given gate chunks (fp8 DR)."""
                nt = int(n_t[t])
                for ci in chunks:
                    Gc, rg = chunk_ref(Gs, ci)
                    ns = slice(ci * 512, (ci + 1) * 512)
                    for j in range(KH // 2):
                        nc.tensor.matmul(Gc[0:nt, rg],
                                         hstage8[:, 2 * j:2 * j + 2, hcol:hcol + nt],
                                         whh8_sb[:, 2 * j:2 * j + 2, ns],
                                         start=False, stop=False, perf_mode=DR)

            # --- spread-out batched output projection ---------------------
            pending = []          # chunk matmuls not yet emitted: (rec, n0, n1)
            deferred = []         # copies not yet emitted: (rec, n0, n1, ps)
            class _Flush:
                __slots__ = ("stage", "lg", "rows", "segments", "left")

            def queue_flush(stage, segments):
                rec = _Flush()
                rec.stage = stage
                rec.segments = segments
                rec.rows = segments[-1][2]
                rec.lg = work.tile([P, VS], BF16, tag="lg", bufs=3, name="lg")
                rec.left = 0
                for n0 in range(0, VS, 512):
                    pending.append((rec, n0, min(n0 + 512, VS)))
                    rec.left += 1

            def emit_chunk_mms():
                """One 512-col output-projection chunk's matmuls; the copy is
                deferred off the critical chain."""
                rec, n0, n1 = pending.pop(0)
                ps = ps_o.tile([P, 512], F32, tag="o512")
                for k in range(KH):
                    nc.tensor.matmul(ps[0:rec.rows, 0:n1 - n0],
                                     rec.stage[:, k, 0:rec.rows],
                                     owt_sb[:, k, n0:n1],
                                     start=(k == 0), stop=(k == KH - 1))
                deferred.append((rec, n0, n1, ps))

            cp_flip = [0]

            def emit_copies():
                """Drain deferred PSUM->SBUF projection copies (ACT/DVE
                alternating); issue the output DMA when a batch completes."""
                while deferred:
                    rec, n0, n1, ps = deferred.pop(0)
                    if cp_flip[0] == 0:
                        nc.scalar.activation(rec.lg[0:rec.rows, n0:n1],
                                             ps[0:rec.rows, 0:n1 - n0], COPY)
                    else:
                        nc.vector.tensor_copy(rec.lg[0:rec.rows, n0:n1],
                                              ps[0:rec.rows, 0:n1 - n0])
                    cp_flip[0] ^= 1
                    rec.left -= 1
                    if rec.left == 0:
                        for (ti_, r0, r1) in rec.segments:
                            nc.sync.dma_start(out[ti_, 0:r1 - r0, :],
                                              rec.lg[r0:r1, :])

            def pointwise_tail(t, t_i, t_g, t_o, first=False, t_f=None):
                """ig2 = (tanh_i+1)*tanh_g (=2*i*g); c2 += ; h2 = (tanh_o+1)*
                tanh(c) (=2h).  Returns h2 (doubled h, bf16)."""
                nt = int(n_t[t])
                r = slice(0, nt)
                ig2 = work.tile([P, H], BF16, tag="ig")
                nc.vector.scalar_tensor_tensor(ig2[r, :], t_i, 1.0, t_g, ADD, MULT)
                if first:
                    nc.vector.tensor_copy(c2_sb[r, :], ig2[r, :])
                else:
                    nc.vector.tensor_add(c2_sb[r, :], ig2[r, :], t_f)
                tc_ = work.tile([P, H], BF16, tag="tanhc")
                nc.scalar.activation(tc_[r, :], c2_sb[r, :], TANH, scale=0.5)
                h2 = work.tile([P, H], BF16, tag="h2")
                nc.vector.scalar_tensor_tensor(h2[r, :], t_o, 1.0, tc_[r, :],
                                               ADD, MULT)
                return h2

            def pointwise_store(t, h2, stage, stage8, col0):
                """PE-transpose h2 (valid rows only) into the stage tiles."""
                nt = int(n_t[t])
                pst = ps_tr.tile([P, 4 * P], BF16, tag="tr")
                pst3 = pst.rearrange("p (m b) -> p m b", m=KH)
                for m in range(KH):
                    nc.tensor.transpose(pst3[:, m, 0:nt],
                                        h2[0:nt, m * P:(m + 1) * P],
                                        ident16[0:nt, 0:nt])
                nc.vector.tensor_copy(stage8[:, :, col0:col0 + nt], pst3[:, :, 0:nt])
                nc.vector.tensor_copy(stage[:, :, col0:col0 + nt], pst3[:, :, 0:nt])

            # ---------------- step 0 (gates precomputed on host) ----------------
            cur, col0 = 0, plan[0][0]
            r0 = slice(0, P)
            t_i0 = work.tile([P, H], BF16, tag="ti")
            nc.scalar.activation(t_i0[r0, :], g0_sb[:, 512:1024], TANH)
            t_g0 = work.tile([P, H], BF16, tag="tg")
            nc.scalar.activation(t_g0[r0, :], g0_sb[:, 0:512], TANH)
            t_o0 = work.tile([P, H], BF16, tag="to")
            nc.scalar.activation(t_o0[r0, :], g0_sb[:, 1536:2048], TANH)
            h2 = pointwise_tail(0, t_i0[r0, :], t_g0[r0, :], t_o0[r0, :],
                                first=True)
            pointwise_store(0, h2, stages[cur], stages8[cur], col0)

            xT_next, x8_next = fetch_x(1)
            S_next = start_scores(1, xT_next)
            G_next = seed_gates(1, x8_next)
            xT_fut = fetch_x(2)

            # ---------------- recurrence ----------------
            prev_stage, prev_stage8, prev_col = stages[cur], stages8[cur], col0
            for t in range(1, T):
                nt = int(n_t[t])
                r = slice(0, nt)
                col0, flush = plan[t]
                S, Gs = S_next, G_next
                xT_next, x8_next = xT_fut

                # --- attention finish + per-m exp/sum, pipelined ---
                sc = work.tile([P, KA, B], BF16, tag="sc")
                trt = ps_tr.tile([P, 512], F32, tag="tr")
                for m in range(MA):
                    for k in range(KH):
                        nc.tensor.matmul(S[:, m, 0:nt],
                                         awh_sb[:, k, m * P:(m + 1) * P],
                                         prev_stage[:, k, prev_col:prev_col + nt],
                                         start=False, stop=(k == KH - 1))
                    nc.scalar.activation(sc[:, m, 0:nt], S[:, m, 0:nt], EXP)
                    if m >= 1:  # sum lags one m-group: no PE stall on exp
                        nc.tensor.matmul(trt[0:1, 0:nt], ones_bf,
                                         sc[:, m - 1, 0:nt],
                                         start=(m == 1), stop=False)
                nc.tensor.matmul(trt[0:1, 0:nt], ones_bf, sc[:, MA - 1, 0:nt],
                                 start=False, stop=True)

                # projection chunk + first h_side half fill the softmax window
                if pending:
                    emit_chunk_mms()
                h_side_half(t, Gs, prev_stage8, prev_col, (CH_F, CH_O))

                rden = work.tile([1, B], F32, tag="rden")
                nc.vector.reciprocal_approx_fast(rden[:, 0:nt], trt[0:1, 0:nt])
                rden_bf = work.tile([1, B], BF16, tag="rdenb")
                nc.vector.tensor_copy(rden_bf[:, 0:nt], rden[:, 0:nt])
                nc.tensor.matmul(trt[:, 128:128 + nt], ones_row.to_broadcast([1, P]),
                                 rden_bf[:, 0:nt], start=True, stop=True)
                h_side_half(t, Gs, prev_stage8, prev_col, (CH_I, CH_G))

                # attn8 = (sc * cnn) * (1/den), in two k-pair halves
                rbc = trt[:, 128:256].rearrange("p (k b) -> p k b", k=1)
                attn8 = work.tile([P, KA, B], F8, tag="attn8")
                for half in range(2):
                    ks = slice(2 * half, 2 * half + 2)
                    atth = work.tile([P, 2, B], BF16, tag=f"att{half}")
                    nc.vector.tensor_mul(atth[:, :, 0:nt], sc[:, ks, 0:nt],
                                         cnn_sb[:, ks, 0:nt])
                    nc.vector.tensor_tensor(
                        attn8[:, ks, 0:nt], atth[:, :, 0:nt],
                        rbc[:, :, 0:nt].to_broadcast([P, 2, nt]), op=MULT)

                # --- att-side GEMMs close chunks [f,o,i,g]; tanh per chunk ---
                tanhs = {}
                for ci in (CH_F, CH_O, CH_I, CH_G):
                    Gc, rg = chunk_ref(Gs, ci)
                    ns = slice(ci * 512, (ci + 1) * 512)
                    for j in range(KA // 2):
                        nc.tensor.matmul(Gc[0:nt, rg],
                                         attn8[:, 2 * j:2 * j + 2, 0:nt],
                                         ca8_sb[:, 2 * j:2 * j + 2, ns],
                                         start=False, stop=(j == KA // 2 - 1),
                                         perf_mode=DR)
                    th = work.tile([P, H], BF16, tag=f"t{ci}")
                    nc.scalar.activation(th[r, :], Gc[r, rg], TANH)
                    tanhs[ci] = th[r, :]
                    if ci == CH_F:
                        sf = work.tile([P, H], BF16, tag="sf")
                        nc.vector.tensor_scalar(sf[r, :], tanhs[CH_F], 1.0, 0.5,
                                                ADD, MULT)
                        fc2 = work.tile([P, H], BF16, tag="fc")
                        nc.vector.tensor_mul(fc2[r, :], sf[r, :], c2_sb[r, :])

                h2 = pointwise_tail(t, tanhs[CH_I], tanhs[CH_G], tanhs[CH_O],
                                    t_f=fc2[r, :])

                # ---- pointwise window: independent PE work + deferred copies ----
                if flush is not None:
                    while pending:
                        emit_chunk_mms()
                    emit_copies()
                    queue_flush(stages[cur], flush)
                    cur ^= 1
                else:
                    if pending:
                        emit_chunk_mms()
                    if pending:
                        emit_chunk_mms()
                    emit_copies()

                if t + 1 < T:
                    G_next = seed_gates(t + 1, x8_next)
                    S_next = start_scores(t + 1, xT_next)
                pointwise_store(t, h2, stages[cur], stages8[cur], col0)
                if t + 2 < T:
                    xT_fut = fetch_x(t + 2)

                prev_stage, prev_stage8 = stages[cur], stages8[cur]
                prev_col = col0

            queue_flush(stages[cur], final_segs)
            while pending:
                emit_chunk_mms()
            emit_copies()

    nc.finalize()
    return nc


def _reorder_gates(w, axis):
    """Reorder the 4H gate dim from [i|f|g|o] (torch order) to [g|i|f|o]."""
    idx = np.concatenate([np.arange(2 * H, 3 * H), np.arange(0, H),
                          np.arange(H, 2 * H), np.arange(3 * H, 4 * H)])
    return np.take(w, idx, axis=axis)


def _prep_inputs(inputs):
    f = {k: np.asarray(v) for k, v in inputs.items()}
    lengths = f["lengths"].astype(np.int64)
    n_t = [int((lengths > t).sum()) for t in range(T)]

    att_W = np.asarray(f["att_W"], np.float32)
    attd_W = np.asarray(f["attd_W"], np.float32)
    W_ih = _reorder_gates(np.asarray(f["W_ih"], np.float32), axis=0)
    W_hh = _reorder_gates(np.asarray(f["W_hh"], np.float32), axis=0)
    b0 = _reorder_gates(np.asarray(f["b_ih"], np.float32)
                        + np.asarray(f["b_hh"], np.float32), axis=0)
    out_W = np.asarray(f["out_W"], np.float32)

    def bf(x):
        return np.ascontiguousarray(x.astype(NP_BF16))

    def f8(x):
        return np.ascontiguousarray(x.astype(NP_F8))

    # host-side fold matrices (fp32) for the fp8 gate GEMMs
    cx = attd_W[:, :E].T @ W_ih.T                     # (E, 4H)
    ca = attd_W[:, E:].T @ W_ih.T                     # (A, 4H)
    bc = np.asarray(f["attd_b"], np.float32) @ W_ih.T + b0   # (4H,)
    g0 = np.asarray(f["features"], np.float32) @ W_ih.T + b0  # (B, 4H)

    # fold the sigmoid half-angle scaling into the i/f/o gate columns
    # (gate order [g|i|f|o]: columns H:4H get 0.5)
    gs = np.ones((G4,), np.float32)
    gs[H:] = 0.5
    cx *= gs
    ca *= gs
    whh_s = W_hh.T * gs
    bc = bc * gs
    g0 = g0 * gs

    # h is stored doubled on the device: halve every weight that consumes h
    base = {
        "cnn_T": bf(np.asarray(f["cnn_features"], np.float32).T),
        "emb_W": bf(np.asarray(f["emb_W"], np.float32)),
        "awh": bf(0.5 * att_W[:, E:].T),
        "awx": bf(att_W[:, :E].T),
        "attb_row": bf(np.asarray(f["att_b"], np.float32).reshape(1, A)),
        "cx8": f8(cx),
        "ca8": f8(ca),
        "whh8": f8(0.5 * whh_s),
        "bc4": bf(np.broadcast_to(bc.reshape(1, G4), (P, G4))),
        "g0": np.ascontiguousarray(g0.astype(np.float32)),
    }

    caps = np.asarray(f["captions"], np.int64)          # (B, T-1)
    caps_pad = np.zeros((T, B), np.int32)
    caps_pad[:T - 1] = caps.T.astype(np.int32)          # caps_pad[t-1] = x_t tokens
    base["caps"] = np.ascontiguousarray(caps_pad)

    in_maps = []
    for c in range(NCORES):
        m = dict(base)
        m["owt"] = bf(0.5 * out_W[c * VS:(c + 1) * VS].T)
        in_maps.append(m)
    return in_maps, n_t


_CACHE = {}


def kernel(**inputs):
    in_maps, n_t = _prep_inputs(inputs)
    key = tuple(n_t)
    if key not in _CACHE:
        _CACHE[key] = _build_nc(n_t)
    nc = _CACHE[key]
    res = run_bass_kernel_spmd(nc, in_maps, list(range(NCORES)))
    outs = [np.asarray(res.results[c]["out"]) for c in range(NCORES)]
    full = np.concatenate(outs, axis=-1).astype(np.float32)   # (T, B, V)
    full += np.asarray(inputs["out_b"], np.float32)[None, None, :]
    # device only writes the first n_t[t] (valid) rows of each step
    mask = np.arange(B)[None, :] < np.asarray(n_t)[:, None]   # (T, B)
    full[~mask] = 0.0
    return full


# revision 21
# speedup vs baseline: 1.2665x; 1.2665x over previous
"""Trainium2 Bass kernel for nn_DecoderRNN (attention LSTM decoder + vocab projection).

Strategy (8 NeuronCores):
  - The 63-step LSTM/attention recurrence is replicated on all cores (identical
    SPMD program); the dominant output projection (T*B, H) x (H, V) is sharded
    over the vocab dimension (V/8 = 1250 logit columns per core). No collectives.
  - fp8-e4m3 + DoubleRow perf mode for the recurrence GEMMs (gates from x/
    attended/h).  Fold matrices Cx/Ca and the step-0 gates are precomputed on
    the host in fp32.
  - Gate columns are ordered [g|i|f|o]; each 512-wide gate chunk's bias is
    seeded by FOUR CONCURRENT K=1 matmuls (tile_position row strips
    0/32/64/96, one PSUM bank each).
  - The recurrent critical path is minimized: attention scores use four
    per-m-tile PSUM tiles so exp/sum pipeline inside the attention-finish
    GEMMs without tile-level WAR serialization; fast approximate reciprocal;
    fp32 broadcast matmul; attention-side GEMMs close gate chunks in
    [f,o,i,g] order so the two fused 1024-wide tanh ops never wait on
    later chunk writes.
  - h is stored DOUBLED (2h): the LSTM tail is computed feature-major --
    tanh_o and tanh(c) are PE-transposed separately and fused via ONE
    scalar_tensor_tensor op ((tanh_o+1)*tanh_c) writing the fp8 h-stage
    directly; the 0.5 is folded into the att_Wh / W_hh / out_W weights on
    the host.  c is stored doubled too (tanh(c) via the activation input
    scale).  The attention-finish GEMM streams the fp8 h-stage.  All
    activations stay on the exp/tanh table: zero table reloads in the loop.
  - The output projection runs on 128-row batches with a double-buffered PSUM
    bank; its PSUM->SBUF copies are deferred off the critical chain.
  - Idle-phase "heater" matmuls keep the PE HAM clock-gate at full rate
    during the initial weight DMA and the chain-bound late (small-batch)
    steps.
  - Logits are written bf16, valid rows only; the host zero-fills, upcasts,
    and adds the output bias.  Ragged lengths are baked into the instruction
    stream.
"""

import os
import sys

import numpy as np

for _p in ("/opt/trn_rl_repo", "/root/.axon_site/_ro/trn_rl_repo"):
    if os.path.isdir(_p) and _p not in sys.path:
        sys.path.insert(0, _p)

import ml_dtypes
import concourse.bass as bass
import concourse.tile as tile
from concourse import bacc, mybir
from concourse.bass_utils import run_bass_kernel_spmd
from concourse.masks import make_identity

F32 = mybir.dt.float32
BF16 = mybir.dt.bfloat16
F8 = mybir.dt.float8e4
I32 = mybir.dt.int32
ADD = mybir.AluOpType.add
MULT = mybir.AluOpType.mult
TANH = mybir.ActivationFunctionType.Tanh
EXP = mybir.ActivationFunctionType.Exp
COPY = mybir.ActivationFunctionType.Copy
DR = mybir.MatmulPerfMode.DoubleRow
NP_BF16 = ml_dtypes.bfloat16
NP_F8 = np.dtype(mybir.dt.np(F8))

B, T, E, H, A, V = 128, 64, 512, 512, 512, 10000
G4 = 4 * H                      # 2048
NCORES = 8
VS = V // NCORES                # 1250 vocab columns per core
P = 128

KE = E // P                     # 4 k-tiles over E
KH = H // P
KA = A // P
MA = A // P                     # A m-tiles (feature-major attention)
NCH = 4                         # four 512-wide gate chunks: [g|i|f|o]
CH_G, CH_I, CH_F, CH_O = 0, 1, 2, 3
HEAT_T0 = 40                    # heater matmuls from this step on


def _flush_plan(n_t):
    """Pack per-step h rows into 128-row batches for the output projection."""
    plan = []          # per t: (col0, flush_before: segments or None)
    segs = []
    pos = 0
    for t in range(T):
        nt = int(n_t[t])
        flush = None
        if pos + nt > P:
            flush = segs
            segs = []
            pos = 0
        plan.append((pos, flush))
        segs.append((t, pos, pos + nt))
        pos += nt
    return plan, segs  # segs = final leftover batch


def _build_nc(n_t):
    nc = bacc.Bacc("TRN2", target_bir_lowering=False, debug=False,
                   num_devices=NCORES)

    # ---------------- I/O ----------------
    cnn_T = nc.declare_dram_parameter("cnn_T", [A, B], BF16, isOutput=False)
    caps = nc.declare_dram_parameter("caps", [T, B], I32, isOutput=False)
    emb_W = nc.declare_dram_parameter("emb_W", [V, E], BF16, isOutput=False)
    awh_d = nc.declare_dram_parameter("awh", [H, A], BF16, isOutput=False)
    awx_d = nc.declare_dram_parameter("awx", [E, A], BF16, isOutput=False)
    attb_row = nc.declare_dram_parameter("attb_row", [1, A], BF16, isOutput=False)
    cx8_d = nc.declare_dram_parameter("cx8", [E, G4], F8, isOutput=False)
    ca8_d = nc.declare_dram_parameter("ca8", [A, G4], F8, isOutput=False)
    whh8_d = nc.declare_dram_parameter("whh8", [H, G4], F8, isOutput=False)
    bc4_d = nc.declare_dram_parameter("bc4", [P, G4], BF16, isOutput=False)
    g0_d = nc.declare_dram_parameter("g0", [B, G4], F32, isOutput=False)
    owt_d = nc.declare_dram_parameter("owt", [H, VS], BF16, isOutput=False)
    out = nc.declare_dram_parameter("out", [T, B, VS], BF16, isOutput=True)

    plan, final_segs = _flush_plan(n_t)

    with tile.TileContext(nc) as tc:
        with (
            tc.tile_pool(name="consts", bufs=1) as consts,
            tc.tile_pool(name="state", bufs=1) as state,
            tc.tile_pool(name="work", bufs=2) as work,
            tc.tile_pool(name="xstream", bufs=3) as xstream,
            tc.tile_pool(name="ps_g", bufs=1, space="PSUM") as ps_g,    # 4 banks
            tc.tile_pool(name="ps_s", bufs=1, space="PSUM") as ps_s,    # 1 bank
            tc.tile_pool(name="ps_tr", bufs=1, space="PSUM") as ps_tr,  # 1 bank
            tc.tile_pool(name="ps_o", bufs=2, space="PSUM") as ps_o,    # 2 banks
        ):
            # ------------- weight / const loads (three HWDGE queues) -------------
            ident16 = consts.tile([P, P], BF16)
            make_identity(nc, ident16)
            ones_bf = consts.tile([P, 1], BF16)
            nc.vector.memset(ones_bf, 1.0)
            ones_f32 = consts.tile([1, 1], F32)
            nc.vector.memset(ones_f32, 1.0)
            heat_sb = consts.tile([P, 512], BF16)
            nc.vector.memset(heat_sb, 0.5)

            def heat():
                """Dense dummy matmul to keep the PE HAM clock-gate warm."""
                hp = ps_o.tile([P, 512], F32, tag="o512")
                nc.tensor.matmul(hp[:, :], ident16, heat_sb[:, :],
                                 start=True, stop=True)

            # pre-warm the PE during the initial weight DMA
            for _ in range(28):
                heat()

            def load3(dst, dram_ap):
                nc.sync.dma_start(dst, dram_ap.rearrange("(k p) n -> p k n", p=P))

            def load3b(dst, dram_ap):
                nc.scalar.dma_start(dst, dram_ap.rearrange("(k p) n -> p k n", p=P))

            def load3c(dst, dram_ap):
                nc.gpsimd.dma_start(dst, dram_ap.rearrange("(k p) n -> p k n", p=P))

            cnn_sb = consts.tile([P, KA, B], BF16)
            load3(cnn_sb, cnn_T[:, :])
            attb_sb = consts.tile([1, A], BF16)
            nc.sync.dma_start(attb_sb, attb_row[:, :])
            bc4_sb = consts.tile([P, G4], BF16)
            nc.gpsimd.dma_start(bc4_sb, bc4_d[:, :])
            g0_sb = consts.tile([P, G4], F32)
            nc.sync.dma_start(g0_sb, g0_d[:, :])

            awh_sb = state.tile([P, KH, A], BF16)
            load3(awh_sb, awh_d[:, :])
            awx_sb = state.tile([P, KE, A], BF16)
            load3(awx_sb, awx_d[:, :])
            cx8_sb = state.tile([P, KE, G4], F8)
            load3b(cx8_sb, cx8_d[:, :])
            ca8_sb = state.tile([P, KA, G4], F8)
            load3b(ca8_sb, ca8_d[:, :])
            whh8_sb = state.tile([P, KH, G4], F8)
            load3b(whh8_sb, whh8_d[:, :])
            owt_sb = state.tile([P, KH, VS], BF16)
            load3c(owt_sb, owt_d[:, :])
            toks = state.tile([B, T], I32)
            nc.gpsimd.dma_start(toks, caps[:, :].rearrange("t b -> b t"))

            # recurrent state (c stored doubled; h stored doubled in stages)
            c2_sb = state.tile([P, H], BF16)
            stages = [state.tile([P, KH, P], BF16, name=f"stage{i}")
                      for i in range(2)]
            stages8 = [state.tile([P, KH, P], F8, name=f"stage8_{i}")
                       for i in range(2)]

            ones_row = ones_bf[0:1, 0:1]

            def chunk_ref(Gs, ci):
                return Gs[ci // 2], slice((ci % 2) * 512, (ci % 2 + 1) * 512)

            # ---------------- helpers ----------------
            def fetch_x(t):
                """Gather x_t embeddings; bf16 [E(part), KE, B] + fp8 cast."""
                xg = xstream.tile([P, E], BF16, tag="xg")
                nc.gpsimd.indirect_dma_start(
                    out=xg, out_offset=None, in_=emb_W[:, :],
                    in_offset=bass.IndirectOffsetOnAxis(ap=toks[:, t - 1:t], axis=0))
                xT = xstream.tile([P, KE, B], BF16, tag="xT")
                nc.sync.dma_start_transpose(xT, xg)
                x8 = xstream.tile([P, KE, B], F8, tag="x8")
                nc.gpsimd.tensor_copy(x8, xT)
                return xT, x8

            def start_scores(t, xT):
                """New PSUM score tile for step t: att_b + PA."""
                nt = int(n_t[t])
                S = ps_s.tile([P, MA, B], F32, tag="att")
                for m in range(MA):
                    nc.tensor.matmul(S[:, m, 0:nt],
                                     attb_sb[0:1, m * P:(m + 1) * P],
                                     ones_row.to_broadcast([1, nt]),
                                     start=True, stop=False)
                    for k in range(KE):
                        nc.tensor.matmul(S[:, m, 0:nt],
                                         awx_sb[:, k, m * P:(m + 1) * P],
                                         xT[:, k, 0:nt], start=False, stop=False)
                return S

            def seed_gates(t, x8):
                """Bias via 4 concurrent row-tiled K=1 matmuls + PX (fp8 DR)."""
                nt = int(n_t[t])
                Gs = [ps_g.tile([P, 1024], F32, tag=f"g{h}", name=f"g{h}")
                      for h in range(2)]
                for ci in range(NCH):
                    Gh, rg = chunk_ref(Gs, ci)
                    rp = 32 * ci
                    nc.tensor.matmul(
                        Gh[0:nt, rg],
                        ones_bf[rp:rp + 1, 0:1].to_broadcast([1, nt]),
                        bc4_sb[rp:rp + 1, ci * 512:(ci + 1) * 512],
                        start=True, stop=False, tile_position=(rp, 0))
                for ci in range(NCH):
                    Gh, rg = chunk_ref(Gs, ci)
                    ns = slice(ci * 512, (ci + 1) * 512)
                    for j in range(KE // 2):
                        nc.tensor.matmul(Gh[0:nt, rg], x8[:, 2 * j:2 * j + 2, 0:nt],
                                         cx8_sb[:, 2 * j:2 * j + 2, ns],
                                         start=False, stop=False, perf_mode=DR)
                return Gs

            def h_side(t, Gs, hstage8, hcol):
                """+= h_{t-1} @ W_hh.T (fp8 DR)."""
                nt = int(n_t[t])
                for ci in (CH_F, CH_O, CH_I, CH_G):
                    Gc, rg = chunk_ref(Gs, ci)
                    ns = slice(ci * 512, (ci + 1) * 512)
                    for j in range(KH // 2):
                        nc.tensor.matmul(Gc[0:nt, rg],
                                         hstage8[:, 2 * j:2 * j + 2, hcol:hcol + nt],
                                         whh8_sb[:, 2 * j:2 * j + 2, ns],
                                         start=False, stop=False, perf_mode=DR)

            # --- spread-out batched output projection ---------------------
            pending = []          # chunk matmuls not yet emitted: (rec, n0, n1)
            deferred = []         # copies not yet emitted: (rec, n0, n1, ps)
            class _Flush:
                __slots__ = ("stage", "lg", "rows", "segments", "left")

            def queue_flush(stage, segments):
                rec = _Flush()
                rec.stage = stage
                rec.segments = segments
                rec.rows = segments[-1][2]
                rec.lg = work.tile([P, VS], BF16, tag="lg", bufs=3, name="lg")
                rec.left = 0
                for n0 in range(0, VS, 512):
                    pending.append((rec, n0, min(n0 + 512, VS)))
                    rec.left += 1

            def emit_chunk_mms():
                rec, n0, n1 = pending.pop(0)
                ps = ps_o.tile([P, 512], F32, tag="o512")
                for k in range(KH):
                    nc.tensor.matmul(ps[0:rec.rows, 0:n1 - n0],
                                     rec.stage[:, k, 0:rec.rows],
                                     owt_sb[:, k, n0:n1],
                                     start=(k == 0), stop=(k == KH - 1))
                deferred.append((rec, n0, n1, ps))

            cp_flip = [0]

            def emit_copies():
                while deferred:
                    rec, n0, n1, ps = deferred.pop(0)
                    if cp_flip[0] == 0:
                        nc.scalar.activation(rec.lg[0:rec.rows, n0:n1],
                                             ps[0:rec.rows, 0:n1 - n0], COPY)
                    else:
                        nc.vector.tensor_copy(rec.lg[0:rec.rows, n0:n1],
                                              ps[0:rec.rows, 0:n1 - n0])
                    cp_flip[0] ^= 1
                    rec.left -= 1
                    if rec.left == 0:
                        for (ti_, r0, r1) in rec.segments:
                            nc.sync.dma_start(out[ti_, 0:r1 - r0, :],
                                              rec.lg[r0:r1, :])

            def store_stage(t, toT_sb, pst, stage, stage8, col0):
                """stage8/stage <- (tanh_o^T + 1) * tanh_c^T (both = 2h)."""
                nt = int(n_t[t])
                toT = toT_sb[:, :, 0:nt]
                tcT = pst[:, KH:2 * KH, 0:nt]
                nc.vector.scalar_tensor_tensor(
                    stage8[:, :, col0:col0 + nt], toT, 1.0, tcT, ADD, MULT)
                nc.vector.scalar_tensor_tensor(
                    stage[:, :, col0:col0 + nt], toT, 1.0, tcT, ADD, MULT)

            # ---------------- step 0 (gates precomputed on host) ----------------
            cur, col0 = 0, plan[0][0]
            r0 = slice(0, P)
            t_i0 = work.tile([P, H], BF16, tag="ti")
            nc.scalar.activation(t_i0[r0, :], g0_sb[:, 512:1024], TANH)
            t_g0 = work.tile([P, H], BF16, tag="tg")
            nc.scalar.activation(t_g0[r0, :], g0_sb[:, 0:512], TANH)
            t_o0 = work.tile([P, 2 * H], BF16, tag="tfo")
            nc.scalar.activation(t_o0[r0, H:2 * H], g0_sb[:, 1536:2048], TANH)
            ig2 = work.tile([P, H], BF16, tag="ig")
            nc.vector.scalar_tensor_tensor(ig2[r0, :], t_i0[r0, :], 1.0,
                                           t_g0[r0, :], ADD, MULT)
            nc.vector.tensor_copy(c2_sb[r0, :], ig2[r0, :])
            tc_ = work.tile([P, H], BF16, tag="tanhc")
            nc.scalar.activation(tc_[r0, :], c2_sb[r0, :], TANH, scale=0.5)
            pst = ps_tr.tile([P, 2 * KH, P], BF16, tag="tr")
            for m in range(KH):
                nc.tensor.transpose(pst[:, m, :],
                                    t_o0[:, H + m * P:H + (m + 1) * P], ident16)
                nc.tensor.transpose(pst[:, KH + m, :],
                                    tc_[:, m * P:(m + 1) * P], ident16)
            toT_sb = work.tile([P, KH, P], BF16, tag="toT")
            nc.vector.tensor_copy(toT_sb[:, :, :], pst[:, 0:KH, :])
            store_stage(0, toT_sb, pst, stages[cur], stages8[cur], col0)

            xT_next, x8_next = fetch_x(1)
            S_next = start_scores(1, xT_next)
            G_next = seed_gates(1, x8_next)
            h_side(1, G_next, stages8[cur], col0)
            xT_fut = fetch_x(2)

            # ---------------- recurrence ----------------
            prev_stage8, prev_col = stages8[cur], col0
            for t in range(1, T):
                nt = int(n_t[t])
                r = slice(0, nt)
                col0, flush = plan[t]
                S, Gs = S_next, G_next
                xT_next, x8_next = xT_fut

                # --- attention finish, fused exp, denominator sums ---
                sc = work.tile([P, KA, B], BF16, tag="sc")
                trt = ps_tr.tile([P, 512], F32, tag="tr")
                for m in range(MA):
                    for k in range(KH):
                        nc.tensor.matmul(S[:, m, 0:nt],
                                         awh_sb[:, k, m * P:(m + 1) * P],
                                         prev_stage8[:, k, prev_col:prev_col + nt],
                                         start=False, stop=(k == KH - 1))
                nc.scalar.activation(sc[:, :, 0:nt], S[:, :, 0:nt], EXP)
                for m in range(MA):
                    nc.tensor.matmul(trt[0:1, 0:nt], ones_bf, sc[:, m, 0:nt],
                                     start=(m == 0), stop=(m == MA - 1))

                if pending:
                    emit_chunk_mms()
                elif t >= HEAT_T0:
                    heat()

                rden = work.tile([1, B], F32, tag="rden")
                nc.vector.reciprocal_approx_fast(rden[:, 0:nt], trt[0:1, 0:nt])
                nc.tensor.matmul(trt[:, 128:128 + nt],
                                 ones_f32.to_broadcast([1, P]),
                                 rden[:, 0:nt], start=True, stop=True)

                # attn8 = (sc * cnn) * (1/den), in two k-pair halves
                rbc = trt[:, 128:256].rearrange("p (k b) -> p k b", k=1)
                att8s = []
                for half in range(2):
                    ks = slice(2 * half, 2 * half + 2)
                    atth = work.tile([P, 2, B], BF16, tag=f"att{half}")
                    nc.vector.tensor_mul(atth[:, :, 0:nt], sc[:, ks, 0:nt],
                                         cnn_sb[:, ks, 0:nt])
                    a8 = work.tile([P, 2, B], F8, tag=f"att8{half}")
                    nc.vector.tensor_tensor(
                        a8[:, :, 0:nt], atth[:, :, 0:nt],
                        rbc[:, :, 0:nt].to_broadcast([P, 2, nt]), op=MULT)
                    att8s.append(a8)

                # --- att-side GEMMs close chunks [f,o,i,g]; fused tanhs ---
                for ci in (CH_F, CH_O, CH_I, CH_G):
                    Gc, rg = chunk_ref(Gs, ci)
                    ns = slice(ci * 512, (ci + 1) * 512)
                    for j in range(KA // 2):
                        nc.tensor.matmul(Gc[0:nt, rg],
                                         att8s[j][:, :, 0:nt],
                                         ca8_sb[:, 2 * j:2 * j + 2, ns],
                                         start=False, stop=(j == KA // 2 - 1),
                                         perf_mode=DR)
                    if ci == CH_O:
                        tfo = work.tile([P, 2 * H], BF16, tag="tfo")
                        nc.scalar.activation(tfo[r, :], Gs[1][r, :], TANH)
                        sf = work.tile([P, H], BF16, tag="sf")
                        nc.vector.tensor_scalar(sf[r, :], tfo[r, 0:H], 1.0, 0.5,
                                                ADD, MULT)
                        fc2 = work.tile([P, H], BF16, tag="fc")
                        nc.vector.tensor_mul(fc2[r, :], sf[r, :], c2_sb[r, :])
                        # transpose tanh_o early (feature-major h tail)
                        pst = ps_tr.tile([P, 2 * KH, P], BF16, tag="tr")
                        for m in range(KH):
                            nc.tensor.transpose(pst[:, m, 0:nt],
                                                tfo[0:nt, H + m * P:H + (m + 1) * P],
                                                ident16[0:nt, 0:nt])
                        toT_sb = work.tile([P, KH, P], BF16, tag="toT")
                        nc.vector.tensor_copy(toT_sb[:, :, 0:nt],
                                              pst[:, 0:KH, 0:nt])
                tgi = work.tile([P, 2 * H], BF16, tag="tgi")
                nc.scalar.activation(tgi[r, :], Gs[0][r, :], TANH)
                ig2 = work.tile([P, H], BF16, tag="ig")
                nc.vector.scalar_tensor_tensor(ig2[r, :], tgi[r, H:2 * H], 1.0,
                                               tgi[r, 0:H], ADD, MULT)
                nc.vector.tensor_add(c2_sb[r, :], fc2[r, :], ig2[r, :])
                tc_ = work.tile([P, H], BF16, tag="tanhc")
                nc.scalar.activation(tc_[r, :], c2_sb[r, :], TANH, scale=0.5)
                for m in range(KH):
                    nc.tensor.transpose(pst[:, KH + m, 0:nt],
                                        tc_[0:nt, m * P:(m + 1) * P],
                                        ident16[0:nt, 0:nt])

                # ---- pointwise window: independent PE work + deferred copies ----
                if flush is not None:
                    while pending:
                        emit_chunk_mms()
                    queue_flush(stages[cur], flush)
                    cur ^= 1
                else:
                    if pending:
                        emit_chunk_mms()
                    if pending:
                        emit_chunk_mms()
                    elif t >= HEAT_T0:
                        heat()
                        heat()
                emit_copies()

                if t + 1 < T:
                    G_next = seed_gates(t + 1, x8_next)
                    S_next = start_scores(t + 1, xT_next)
                store_stage(t, toT_sb, pst, stages[cur], stages8[cur], col0)
                if t + 1 < T:
                    h_side(t + 1, G_next, stages8[cur], col0)
                if t + 2 < T:
                    xT_fut = fetch_x(t + 2)

                prev_stage8, prev_col = stages8[cur], col0

            queue_flush(stages[cur], final_segs)
            while pending:
                emit_chunk_mms()
            emit_copies()

    nc.finalize()
    return nc


def _reorder_gates(w, axis):
    """Reorder the 4H gate dim from [i|f|g|o] (torch order) to [g|i|f|o]."""
    idx = np.concatenate([np.arange(2 * H, 3 * H), np.arange(0, H),
                          np.arange(H, 2 * H), np.arange(3 * H, 4 * H)])
    return np.take(w, idx, axis=axis)


def _prep_inputs(inputs):
    f = {k: np.asarray(v) for k, v in inputs.items()}
    lengths = f["lengths"].astype(np.int64)
    n_t = [int((lengths > t).sum()) for t in range(T)]

    att_W = np.asarray(f["att_W"], np.float32)
    attd_W = np.asarray(f["attd_W"], np.float32)
    W_ih = _reorder_gates(np.asarray(f["W_ih"], np.float32), axis=0)
    W_hh = _reorder_gates(np.asarray(f["W_hh"], np.float32), axis=0)
    b0 = _reorder_gates(np.asarray(f["b_ih"], np.float32)
                        + np.asarray(f["b_hh"], np.float32), axis=0)
    out_W = np.asarray(f["out_W"], np.float32)

    def bf(x):
        return np.ascontiguousarray(x.astype(NP_BF16))

    def f8(x):
        return np.ascontiguousarray(x.astype(NP_F8))

    # host-side fold matrices (fp32) for the fp8 gate GEMMs
    cx = attd_W[:, :E].T @ W_ih.T                     # (E, 4H)
    ca = attd_W[:, E:].T @ W_ih.T                     # (A, 4H)
    bc = np.asarray(f["attd_b"], np.float32) @ W_ih.T + b0   # (4H,)
    g0 = np.asarray(f["features"], np.float32) @ W_ih.T + b0  # (B, 4H)

    # fold the sigmoid half-angle scaling into the i/f/o gate columns
    # (gate order [g|i|f|o]: columns H:4H get 0.5)
    gs = np.ones((G4,), np.float32)
    gs[H:] = 0.5
    cx *= gs
    ca *= gs
    whh_s = W_hh.T * gs
    bc = bc * gs
    g0 = g0 * gs

    # h is stored doubled on the device: halve every weight that consumes h
    base = {
        "cnn_T": bf(np.asarray(f["cnn_features"], np.float32).T),
        "emb_W": bf(np.asarray(f["emb_W"], np.float32)),
        "awh": bf(0.5 * att_W[:, E:].T),
        "awx": bf(att_W[:, :E].T),
        "attb_row": bf(np.asarray(f["att_b"], np.float32).reshape(1, A)),
        "cx8": f8(cx),
        "ca8": f8(ca),
        "whh8": f8(0.5 * whh_s),
        "bc4": bf(np.broadcast_to(bc.reshape(1, G4), (P, G4))),
        "g0": np.ascontiguousarray(g0.astype(np.float32)),
    }

    caps = np.asarray(f["captions"], np.int64)          # (B, T-1)
    caps_pad = np.zeros((T, B), np.int32)
    caps_pad[:T - 1] = caps.T.astype(np.int32)          # caps_pad[t-1] = x_t tokens
    base["caps"] = np.ascontiguousarray(caps_pad)

    in_maps = []
    for c in range(NCORES):
        m = dict(base)
        m["owt"] = bf(0.5 * out_W[c * VS:(c + 1) * VS].T)
        in_maps.append(m)
    return in_maps, n_t


_CACHE = {}


def kernel(**inputs):
    in_maps, n_t = _prep_inputs(inputs)
    key = tuple(n_t)
    if key not in _CACHE:
        _CACHE[key] = _build_nc(n_t)
    nc = _CACHE[key]
    res = run_bass_kernel_spmd(nc, in_maps, list(range(NCORES)))
    outs = [np.asarray(res.results[c]["out"]) for c in range(NCORES)]
    full = np.concatenate(outs, axis=-1).astype(np.float32)   # (T, B, V)
    full += np.asarray(inputs["out_b"], np.float32)[None, None, :]
    # device only writes the first n_t[t] (valid) rows of each step
    mask = np.arange(B)[None, :] < np.asarray(n_t)[:, None]   # (T, B)
    full[~mask] = 0.0
    return full
